# revision 41
# baseline (speedup 1.0000x reference)
"""ReLU-attention (AttentionMobile) Trainium2 Bass kernel.

Reference computation (fp32):
    q  = x @ Wq ; kv = x @ Wkv ; k = v = kv          (per batch, [S, D])
    per head h (Dh=64): A = relu(q_h k_h^T / sqrt(Dh)); o_h = A v_h
    out = concat_h(o_h) @ Wout + bout

Sharding: batch*heads across 8 cores — core c = (b, head-block j) with
b = c // 4, j = c % 4; each core owns 4 heads (256 cols of Wq/Wkv, 256 rows
of Wout) of one batch and computes a partial [S, D] output; host sums the 4
partials per batch and adds bout.

On-core dataflow (all matmuls bf16 with fp32 PSUM accumulation):
    xT [D, S] (host-pretransposed) -> SBUF; QT/KT pair tiles [128, S] from
    weight-stationary matmuls (1/sqrt(Dh) folded into Wq on host); V t-tiles
    [128, 256] from x-stationary matmuls.

    Attention is software-pipelined over 8 (chunk, head) slots: in slot i the
    PE stream interleaves, per t-tile, the scores matmul pair of head_i with
    the A@V accumulation pair of head_{i-1}, so the PE never waits for the
    relu engines. relu halves go to ACT and DVE simultaneously. The output
    projection of a finished chunk is drip-fed one PSUM group per t-tile into
    the next slot.
"""

import os
import sys

import numpy as np
import ml_dtypes

for _p in ("/opt/trn_rl_repo",):
    if os.path.isdir(_p) and _p not in sys.path:
        sys.path.insert(0, _p)

B, S, D = 2, 2048, 1024
HEADS, DH = 16, 64
N_CORES = 8
HPC = HEADS * B // N_CORES          # heads per core = 4
HC = HPC * DH                       # per-core head cols = 256
KT_N = D // 128                     # 8 contraction tiles for projections
TT_N = S // 128                     # 16 t-tiles
SC_W = 512                          # matmul free-dim (one PSUM bank)
AC_W = 1024                         # attention s-chunk width
AC_N = S // AC_W                    # 2 attention chunks

_CACHE = {}
LAST_RESULTS = None


def build_nc():
    """Build and compile the single-core SPMD Bass program."""
    import concourse.mybir as mybir
    import concourse.tile as tile
    from concourse import bacc
    from concourse.bass import ts, ds

    from concourse.masks import make_identity

    f32 = mybir.dt.float32
    bf16 = mybir.dt.bfloat16
    Relu = mybir.ActivationFunctionType.Relu
    Copy = mybir.ActivationFunctionType.Copy

    nc = bacc.Bacc("TRN2", target_bir_lowering=False, debug=False)

    xT_d = nc.dram_tensor("xT", (D, S), bf16, kind="ExternalInput")
    wq_d = nc.dram_tensor("wq", (D, HC), bf16, kind="ExternalInput")
    wkv_d = nc.dram_tensor("wkv", (D, HC), bf16, kind="ExternalInput")
    wout_d = nc.dram_tensor("wout", (HC, D), bf16, kind="ExternalInput")
    part_d = nc.dram_tensor("part", (S, D), f32, kind="ExternalOutput")

    with tile.TileContext(nc) as tc:
        with (
            tc.tile_pool(name="persist", bufs=1) as pp,
            tc.tile_pool(name="at", bufs=34) as atp,
            tc.tile_pool(name="osb", bufs=3) as outp,
        ):
            xt = pp.tile([128, KT_N, S], bf16)       # x.T, d on partitions
            wq = pp.tile([128, KT_N, HC], bf16)
            wkv = pp.tile([128, KT_N, HC], bf16)
            wout = pp.tile([128, 2, D], bf16)
            qt = pp.tile([128, 2, S], bf16)          # per pair: [2 heads*64, S]
            kt = pp.tile([128, 2, S], bf16)
            vt = pp.tile([128, TT_N, HC], bf16)      # t on partitions
            ot = pp.tile([128, 2, S], bf16)          # attention out, dh on part
            ident = pp.tile([128, 128], bf16)
            make_identity(nc, ident[:])

            # Contiguous k-major loads: each projection chain steps as its
            # k-tile lands. Weights go on the ACT HWDGE ring, xt alternates
            # rings, so per-dma issue overhead is paid in parallel.
            for k in range(KT_N):
                nc.gpsimd.dma_start(wq[:, k, :], wq_d[ts(k, 128), :])
                nc.gpsimd.dma_start(wkv[:, k, :], wkv_d[ts(k, 128), :])
                eng = nc.sync if k % 2 == 0 else nc.scalar
                eng.dma_start(xt[:, k, :], xT_d[ts(k, 128), :])
            for p in range(2):
                nc.gpsimd.dma_start(wout[:, p, :], wout_d[ts(p, 128), :])

            with tc.tile_pool(name="psS", bufs=4, space="PSUM") as psS:
              # ---- phase 1: projections, column-chunk-major ----
              with tc.tile_pool(name="psP", bufs=4, space="PSUM") as psP:

                def w_chain(w_sb, out_sb, eng, p, c):
                    ps = psP.tile([128, SC_W], f32, tag="m")
                    for k in range(KT_N):
                        nc.tensor.matmul(
                            ps[:],
                            w_sb[:, k, ts(p, 128)],
                            xt[:, k, ds(c * SC_W, SC_W)],
                            start=(k == 0),
                            stop=(k == KT_N - 1),
                        )
                    if eng == 0:
                        nc.scalar.activation(
                            out_sb[:, p, ds(c * SC_W, SC_W)], ps[:], Copy
                        )
                    else:
                        nc.vector.tensor_copy(out_sb[:, p, ds(c * SC_W, SC_W)], ps[:])

                def v_transpose(tt, p):
                    # v == k: build V t-tiles by PE-transposing kt (exact).
                    pst = psP.tile([128, 128], bf16, tag="m", name="pst")
                    nc.tensor.transpose(pst[:], kt[:, p, ts(tt, 128)], ident[:])
                    if (tt + p) % 2 == 0:
                        nc.scalar.activation(vt[:, tt, ts(p, 128)], pst[:], Copy)
                    else:
                        nc.vector.tensor_copy(vt[:, tt, ts(p, 128)], pst[:])

                for c in range(S // SC_W):
                    w_chain(wkv, kt, 1, 0, c)
                    w_chain(wq, qt, 0, 0, c)
                    w_chain(wkv, kt, 1, 1, c)
                    w_chain(wq, qt, 0, 1, c)
                    for j in range(4):
                        tt = 4 * c + j
                        v_transpose(tt, 0)
                        v_transpose(tt, 1)

              # ---- phase 2: software-pipelined attention ----
              # Chunks of the s-axis; the last chunk is split so the final
              # (non-overlappable) A@V + out-proj drain is short.
              with (
                tc.tile_pool(name="psO", bufs=2, space="PSUM") as psO,
                tc.tile_pool(name="psF", bufs=2, space="PSUM") as psF,
              ):
                chunks = [(0, 1024), (1024, 512), (1536, 256), (1792, 256)]
                heads = [(ci, h) for ci in range(len(chunks)) for h in range(HPC)]
                state = None       # head currently in its A@V stage
                outproj = []       # pending (chunk, s-tile, nch) psum groups

                def emit_outproj_group(ci, st, nch, pool=None, tag="f"):
                    st0 = chunks[ci][0] + st * 128
                    psf = (pool or psF).tile([128, SC_W], f32, tag=tag, name="psf")
                    for p in range(2):
                        nc.tensor.matmul(
                            psf[:],
                            ot[:, p, ds(st0, 128)],
                            wout[:, p, ds(nch * SC_W, SC_W)],
                            start=(p == 0),
                            stop=(p == 1),
                        )
                    if nch == 0:
                        osb = outp.tile([128, D], f32, tag="osb", name="osb")
                        osb_tiles[st0] = osb
                    else:
                        osb = osb_tiles.pop(st0)
                    # split the psum->sbuf copy across both engines so the
                    # psf slot frees in half the time
                    half = SC_W // 2
                    o0 = nch * SC_W
                    nc.scalar.activation(osb[:, ds(o0, half)], psf[:, ds(0, half)], Copy)
                    nc.vector.tensor_copy(
                        osb[:, ds(o0 + half, half)], psf[:, ds(half, half)]
                    )
                    if nch == 1:
                        nc.sync.dma_start(part_d[ds(st0, 128), :], osb[:])

                osb_tiles = {}
                for i in range(len(heads) + 1):
                    cur = heads[i] if i < len(heads) else None
                    if cur is not None:
                        ci, h = cur
                        s0, cw = chunks[ci]
                        ws = [SC_W] * (cw // SC_W) if cw >= SC_W else [cw]
                        nsh = len(ws)
                        p, half = divmod(h, 2)
                        r0 = half * 64
                        ats = []
                    if state is not None:
                        s_s0, s_cw = chunks[state["ci"]]
                        s_ws = [SC_W] * (s_cw // SC_W) if s_cw >= SC_W else [s_cw]
                        s_nsh = len(s_ws)
                        pso = [
                            psO.tile([64, s_ws[sh]], f32, tag="o", name=f"pso{sh}")
                            for sh in range(s_nsh)
                        ]
                    # t-tile blocks: a burst of scores matmuls (fills the 4
                    # psS slots) then a burst of A@V matmuls — minimizes the
                    # PE array-reconfig switches vs per-t-tile interleave.
                    BL = (4 // nsh) if cur is not None else 4
                    for bt in range(0, TT_N, BL):
                        if cur is not None:
                            for tt in range(bt, bt + BL):
                                psas = []
                                for sh in range(nsh):
                                    psa = psS.tile([128, ws[sh]], f32, tag="s")
                                    nc.tensor.matmul(
                                        psa[:],
                                        kt[r0 : r0 + 64, p, ts(tt, 128)],
                                        qt[r0 : r0 + 64, p, ds(s0 + sh * SC_W, ws[sh])],
                                        start=True,
                                        stop=True,
                                    )
                                    psas.append(psa)
                                at = atp.tile([128, cw], bf16, tag="at")
                                for sh in range(nsh):
                                    dst = at[:, ds(sh * SC_W, ws[sh])]
                                    if (sh + tt) % 2 == 0:
                                        nc.scalar.activation(dst, psas[sh][:], Relu)
                                    else:
                                        nc.vector.tensor_scalar_max(
                                            dst, psas[sh][:], 0.0
                                        )
                                ats.append(at)
                        if state is not None:
                            sh_h = state["h"]
                            for tt in range(bt, bt + BL):
                                for sh in range(s_nsh):
                                    nc.tensor.matmul(
                                        pso[sh][:],
                                        vt[:, tt, ds(sh_h * DH, DH)],
                                        state["ats"][tt][:, ds(sh * SC_W, s_ws[sh])],
                                        start=(tt == 0),
                                        stop=(tt == TT_N - 1),
                                    )
                        if outproj:
                            emit_outproj_group(*outproj.pop(0))
                    if state is not None:
                        sh_h = state["h"]
                        sp, shalf = divmod(sh_h, 2)
                        sr0 = shalf * 64
                        for sh in range(s_nsh):
                            dst = ot[sr0 : sr0 + 64, sp, ds(s_s0 + sh * SC_W, s_ws[sh])]
                            if (sh + sh_h) % 2 == 0:
                                nc.scalar.activation(dst, pso[sh][:], Copy)
                            else:
                                nc.vector.tensor_copy(dst, pso[sh][:])
                        if sh_h == HPC - 1:
                            for st in range(s_cw // 128):
                                for nch in range(2):
                                    outproj.append((state["ci"], st, nch))
                    if cur is not None:
                        state = {"ci": ci, "h": h, "ats": ats}
                    else:
                        state = None
                # drain the last chunk's output projection; scores are done,
                # so alternate between psF and the freed psS slots
                drain_i = 0
                while outproj:
                    if drain_i % 2 == 0:
                        emit_outproj_group(*outproj.pop(0))
                    else:
                        emit_outproj_group(*outproj.pop(0), pool=psS, tag="s")
                    drain_i += 1

    nc.compile()
    return nc


def _get_nc():
    if "nc" not in _CACHE:
        _CACHE["nc"] = build_nc()
    return _CACHE["nc"]


def make_in_maps(hidden_states, Wq, Wkv, Wout):
    bf = ml_dtypes.bfloat16
    x = np.asarray(hidden_states, dtype=np.float32)
    Wq = np.asarray(Wq, dtype=np.float32)
    Wkv = np.asarray(Wkv, dtype=np.float32)
    Wout = np.asarray(Wout, dtype=np.float32)
    scale = 1.0 / np.sqrt(np.float32(DH))
    xT = [np.ascontiguousarray(x[b].T).astype(bf) for b in range(B)]
    in_maps = []
    for c in range(N_CORES):
        b, j = divmod(c, N_CORES // B)
        h0 = j * HC
        in_maps.append(
            {
                "xT": xT[b],
                "wq": np.ascontiguousarray(Wq[:, h0 : h0 + HC] * scale).astype(bf),
                "wkv": np.ascontiguousarray(Wkv[:, h0 : h0 + HC]).astype(bf),
                "wout": np.ascontiguousarray(Wout[h0 : h0 + HC, :]).astype(bf),
            }
        )
    return in_maps


def kernel(**inputs):
    global LAST_RESULTS
    from concourse.bass_utils import run_bass_kernel_spmd

    nc = _get_nc()
    in_maps = make_in_maps(
        inputs["hidden_states"], inputs["Wq"], inputs["Wkv"], inputs["Wout"]
    )
    trace = bool(os.environ.get("KERNEL_TRACE"))
    res = run_bass_kernel_spmd(
        nc, in_maps, core_ids=list(range(N_CORES)), trace=trace
    )
    LAST_RESULTS = res
    out = np.zeros((B, S, D), dtype=np.float32)
    for c in range(N_CORES):
        out[c // (N_CORES // B)] += res.results[c]["part"]
    out += np.asarray(inputs["bout"], dtype=np.float32)[None, None, :]
    return out


# revision 43
# speedup vs baseline: 1.0381x; 1.0381x over previous
"""ReLU-attention (AttentionMobile) Trainium2 Bass kernel.

Reference computation (fp32):
    q  = x @ Wq ; kv = x @ Wkv ; k = v = kv          (per batch, [S, D])
    per head h (Dh=64): A = relu(q_h k_h^T / sqrt(Dh)); o_h = A v_h
    out = concat_h(o_h) @ Wout + bout

Sharding: batch*heads across 8 cores — core c = (b, head-block j) with
b = c // 4, j = c % 4; each core owns 4 heads (256 cols of Wq/Wkv, 256 rows
of Wout) of one batch and computes a partial [S, D] output; host sums the 4
partials per batch and adds bout.

On-core dataflow (all matmuls bf16 with fp32 PSUM accumulation):
    xT [D, S] (host-pretransposed) -> SBUF; QT/KT pair tiles [128, S] from
    weight-stationary matmuls (1/sqrt(Dh) folded into Wq on host); V t-tiles
    [128, 256] from x-stationary matmuls.

    Attention is software-pipelined over 8 (chunk, head) slots: in slot i the
    PE stream interleaves, per t-tile, the scores matmul pair of head_i with
    the A@V accumulation pair of head_{i-1}, so the PE never waits for the
    relu engines. relu halves go to ACT and DVE simultaneously. The output
    projection of a finished chunk is drip-fed one PSUM group per t-tile into
    the next slot.
"""

import os
import sys

import numpy as np
import ml_dtypes

for _p in ("/opt/trn_rl_repo",):
    if os.path.isdir(_p) and _p not in sys.path:
        sys.path.insert(0, _p)

B, S, D = 2, 2048, 1024
HEADS, DH = 16, 64
N_CORES = 8
HPC = HEADS * B // N_CORES          # heads per core = 4
HC = HPC * DH                       # per-core head cols = 256
KT_N = D // 128                     # 8 contraction tiles for projections
TT_N = S // 128                     # 16 t-tiles
SC_W = 512                          # matmul free-dim (one PSUM bank)
AC_W = 1024                         # attention s-chunk width
AC_N = S // AC_W                    # 2 attention chunks

_CACHE = {}
LAST_RESULTS = None


def build_nc():
    """Build and compile the single-core SPMD Bass program."""
    import concourse.mybir as mybir
    import concourse.tile as tile
    from concourse import bacc
    from concourse.bass import ts, ds

    from concourse.masks import make_identity

    f32 = mybir.dt.float32
    bf16 = mybir.dt.bfloat16
    Relu = mybir.ActivationFunctionType.Relu
    Copy = mybir.ActivationFunctionType.Copy

    nc = bacc.Bacc("TRN2", target_bir_lowering=False, debug=False)

    xT_d = nc.dram_tensor("xT", (D, S), bf16, kind="ExternalInput")
    wq_d = nc.dram_tensor("wq", (D, HC), bf16, kind="ExternalInput")
    wkv_d = nc.dram_tensor("wkv", (D, HC), bf16, kind="ExternalInput")
    wout_d = nc.dram_tensor("wout", (HC, D), bf16, kind="ExternalInput")
    part_d = nc.dram_tensor("part", (S, D), f32, kind="ExternalOutput")

    with tile.TileContext(nc) as tc:
        with (
            tc.tile_pool(name="persist", bufs=1) as pp,
            tc.tile_pool(name="at", bufs=34) as atp,
            tc.tile_pool(name="osb", bufs=3) as outp,
        ):
            xt = pp.tile([128, KT_N, S], bf16)       # x.T, d on partitions
            wq = pp.tile([128, KT_N, HC], bf16)
            wkv = pp.tile([128, KT_N, HC], bf16)
            wout = pp.tile([128, 2, D], bf16)
            qt = pp.tile([128, 2, S], bf16)          # per pair: [2 heads*64, S]
            kt = pp.tile([128, 2, S], bf16)
            vt = pp.tile([128, TT_N, HC], bf16)      # t on partitions
            ot = pp.tile([128, 2, S], bf16)          # attention out, dh on part
            ident = pp.tile([128, 128], bf16)
            make_identity(nc, ident[:])

            # Contiguous k-major loads: each projection chain steps as its
            # k-tile lands. Weights go on the ACT HWDGE ring, xt alternates
            # rings, so per-dma issue overhead is paid in parallel.
            for k in range(KT_N):
                nc.gpsimd.dma_start(wq[:, k, :], wq_d[ts(k, 128), :])
                nc.gpsimd.dma_start(wkv[:, k, :], wkv_d[ts(k, 128), :])
                eng = nc.sync if k % 2 == 0 else nc.scalar
                eng.dma_start(xt[:, k, :], xT_d[ts(k, 128), :])
            for p in range(2):
                nc.gpsimd.dma_start(wout[:, p, :], wout_d[ts(p, 128), :])

            with tc.tile_pool(name="psS", bufs=4, space="PSUM") as psS:
              # ---- phase 1: projections, column-chunk-major ----
              with tc.tile_pool(name="psP", bufs=4, space="PSUM") as psP:

                def w_chain(w_sb, out_sb, eng, p, c):
                    ps = psP.tile([128, SC_W], f32, tag="m")
                    for k in range(KT_N):
                        nc.tensor.matmul(
                            ps[:],
                            w_sb[:, k, ts(p, 128)],
                            xt[:, k, ds(c * SC_W, SC_W)],
                            start=(k == 0),
                            stop=(k == KT_N - 1),
                        )
                    if eng == 0:
                        nc.scalar.activation(
                            out_sb[:, p, ds(c * SC_W, SC_W)], ps[:], Copy
                        )
                    else:
                        nc.vector.tensor_copy(out_sb[:, p, ds(c * SC_W, SC_W)], ps[:])

                def v_transpose(tt, p):
                    # v == k: build V t-tiles by PE-transposing kt (exact).
                    pst = psP.tile([128, 128], bf16, tag="m", name="pst")
                    nc.tensor.transpose(pst[:], kt[:, p, ts(tt, 128)], ident[:])
                    if (tt + p) % 2 == 0:
                        nc.scalar.activation(vt[:, tt, ts(p, 128)], pst[:], Copy)
                    else:
                        nc.vector.tensor_copy(vt[:, tt, ts(p, 128)], pst[:])

                for c in range(S // SC_W):
                    w_chain(wkv, kt, 1, 0, c)
                    w_chain(wq, qt, 0, 0, c)
                    w_chain(wkv, kt, 1, 1, c)
                    w_chain(wq, qt, 0, 1, c)
                    for j in range(4):
                        tt = 4 * c + j
                        v_transpose(tt, 0)
                        v_transpose(tt, 1)

              # ---- phase 2: software-pipelined attention ----
              # Chunks of the s-axis; the last chunk is split so the final
              # (non-overlappable) A@V + out-proj drain is short.
              with (
                tc.tile_pool(name="psO", bufs=2, space="PSUM") as psO,
                tc.tile_pool(name="psF", bufs=2, space="PSUM") as psF,
              ):
                chunks = [(0, 1024), (1024, 512), (1536, 512)]
                heads = [(ci, h) for ci in range(len(chunks)) for h in range(HPC)]
                state = None       # head currently in its A@V stage
                outproj = []       # pending (chunk, s-tile, nch) psum groups

                def emit_outproj_group(ci, st, nch, pool=None, tag="f"):
                    st0 = chunks[ci][0] + st * 128
                    psf = (pool or psF).tile([128, SC_W], f32, tag=tag, name="psf")
                    for p in range(2):
                        nc.tensor.matmul(
                            psf[:],
                            ot[:, p, ds(st0, 128)],
                            wout[:, p, ds(nch * SC_W, SC_W)],
                            start=(p == 0),
                            stop=(p == 1),
                        )
                    if nch == 0:
                        osb = outp.tile([128, D], f32, tag="osb", name="osb")
                        osb_tiles[st0] = osb
                    else:
                        osb = osb_tiles.pop(st0)
                    # split the psum->sbuf copy across both engines so the
                    # psf slot frees in half the time
                    half = SC_W // 2
                    o0 = nch * SC_W
                    nc.scalar.activation(osb[:, ds(o0, half)], psf[:, ds(0, half)], Copy)
                    nc.vector.tensor_copy(
                        osb[:, ds(o0 + half, half)], psf[:, ds(half, half)]
                    )
                    if nch == 1:
                        nc.sync.dma_start(part_d[ds(st0, 128), :], osb[:])

                osb_tiles = {}
                for i in range(len(heads) + 1):
                    cur = heads[i] if i < len(heads) else None
                    if cur is not None:
                        ci, h = cur
                        s0, cw = chunks[ci]
                        ws = [SC_W] * (cw // SC_W) if cw >= SC_W else [cw]
                        nsh = len(ws)
                        p, half = divmod(h, 2)
                        r0 = half * 64
                        ats = []
                    if state is not None:
                        s_s0, s_cw = chunks[state["ci"]]
                        s_ws = [SC_W] * (s_cw // SC_W) if s_cw >= SC_W else [s_cw]
                        s_nsh = len(s_ws)
                        pso = [
                            psO.tile([64, s_ws[sh]], f32, tag="o", name=f"pso{sh}")
                            for sh in range(s_nsh)
                        ]
                    # t-tile blocks: a burst of scores matmuls (fills the 4
                    # psS slots) then a burst of A@V matmuls — minimizes the
                    # PE array-reconfig switches vs per-t-tile interleave.
                    BL = (4 // nsh) if cur is not None else 4
                    for bt in range(0, TT_N, BL):
                        if cur is not None:
                            for tt in range(bt, bt + BL):
                                psas = []
                                for sh in range(nsh):
                                    psa = psS.tile([128, ws[sh]], f32, tag="s")
                                    nc.tensor.matmul(
                                        psa[:],
                                        kt[r0 : r0 + 64, p, ts(tt, 128)],
                                        qt[r0 : r0 + 64, p, ds(s0 + sh * SC_W, ws[sh])],
                                        start=True,
                                        stop=True,
                                    )
                                    psas.append(psa)
                                at = atp.tile([128, cw], bf16, tag="at")
                                for sh in range(nsh):
                                    dst = at[:, ds(sh * SC_W, ws[sh])]
                                    if (sh + tt) % 2 == 0:
                                        nc.scalar.activation(dst, psas[sh][:], Relu)
                                    else:
                                        nc.vector.tensor_scalar_max(
                                            dst, psas[sh][:], 0.0
                                        )
                                ats.append(at)
                        if state is not None:
                            sh_h = state["h"]
                            for tt in range(bt, bt + BL):
                                for sh in range(s_nsh):
                                    nc.tensor.matmul(
                                        pso[sh][:],
                                        vt[:, tt, ds(sh_h * DH, DH)],
                                        state["ats"][tt][:, ds(sh * SC_W, s_ws[sh])],
                                        start=(tt == 0),
                                        stop=(tt == TT_N - 1),
                                    )
                        if outproj:
                            emit_outproj_group(*outproj.pop(0))
                    if state is not None:
                        sh_h = state["h"]
                        sp, shalf = divmod(sh_h, 2)
                        sr0 = shalf * 64
                        for sh in range(s_nsh):
                            dst = ot[sr0 : sr0 + 64, sp, ds(s_s0 + sh * SC_W, s_ws[sh])]
                            if (sh + sh_h) % 2 == 0:
                                nc.scalar.activation(dst, pso[sh][:], Copy)
                            else:
                                nc.vector.tensor_copy(dst, pso[sh][:])
                        if sh_h == HPC - 1:
                            for st in range(s_cw // 128):
                                for nch in range(2):
                                    outproj.append((state["ci"], st, nch))
                    if cur is not None:
                        state = {"ci": ci, "h": h, "ats": ats}
                    else:
                        state = None
                # drain the last chunk's output projection
                while outproj:
                    emit_outproj_group(*outproj.pop(0))

    nc.compile()
    return nc


def _get_nc():
    if "nc" not in _CACHE:
        _CACHE["nc"] = build_nc()
    return _CACHE["nc"]


def make_in_maps(hidden_states, Wq, Wkv, Wout):
    bf = ml_dtypes.bfloat16
    x = np.asarray(hidden_states, dtype=np.float32)
    Wq = np.asarray(Wq, dtype=np.float32)
    Wkv = np.asarray(Wkv, dtype=np.float32)
    Wout = np.asarray(Wout, dtype=np.float32)
    scale = 1.0 / np.sqrt(np.float32(DH))
    xT = [np.ascontiguousarray(x[b].T).astype(bf) for b in range(B)]
    in_maps = []
    for c in range(N_CORES):
        b, j = divmod(c, N_CORES // B)
        h0 = j * HC
        in_maps.append(
            {
                "xT": xT[b],
                "wq": np.ascontiguousarray(Wq[:, h0 : h0 + HC] * scale).astype(bf),
                "wkv": np.ascontiguousarray(Wkv[:, h0 : h0 + HC]).astype(bf),
                "wout": np.ascontiguousarray(Wout[h0 : h0 + HC, :]).astype(bf),
            }
        )
    return in_maps


def kernel(**inputs):
    global LAST_RESULTS
    from concourse.bass_utils import run_bass_kernel_spmd

    nc = _get_nc()
    in_maps = make_in_maps(
        inputs["hidden_states"], inputs["Wq"], inputs["Wkv"], inputs["Wout"]
    )
    trace = bool(os.environ.get("KERNEL_TRACE"))
    res = run_bass_kernel_spmd(
        nc, in_maps, core_ids=list(range(N_CORES)), trace=trace
    )
    LAST_RESULTS = res
    out = np.zeros((B, S, D), dtype=np.float32)
    for c in range(N_CORES):
        out[c // (N_CORES // B)] += res.results[c]["part"]
    out += np.asarray(inputs["bout"], dtype=np.float32)[None, None, :]
    return out


# revision 44
# speedup vs baseline: 1.0446x; 1.0063x over previous
"""ReLU-attention (AttentionMobile) Trainium2 Bass kernel.

Reference computation (fp32):
    q  = x @ Wq ; kv = x @ Wkv ; k = v = kv          (per batch, [S, D])
    per head h (Dh=64): A = relu(q_h k_h^T / sqrt(Dh)); o_h = A v_h
    out = concat_h(o_h) @ Wout + bout

Sharding: batch*heads across 8 cores — core c = (b, head-block j) with
b = c // 4, j = c % 4; each core owns 4 heads (256 cols of Wq/Wkv, 256 rows
of Wout) of one batch and computes a partial [S, D] output; host sums the 4
partials per batch and adds bout.

On-core dataflow (all matmuls bf16 with fp32 PSUM accumulation):
    xT [D, S] (host-pretransposed) -> SBUF; QT/KT pair tiles [128, S] from
    weight-stationary matmuls (1/sqrt(Dh) folded into Wq on host); V t-tiles
    [128, 256] from x-stationary matmuls.

    Attention is software-pipelined over 8 (chunk, head) slots: in slot i the
    PE stream interleaves, per t-tile, the scores matmul pair of head_i with
    the A@V accumulation pair of head_{i-1}, so the PE never waits for the
    relu engines. relu halves go to ACT and DVE simultaneously. The output
    projection of a finished chunk is drip-fed one PSUM group per t-tile into
    the next slot.
"""

import os
import sys

import numpy as np
import ml_dtypes

for _p in ("/opt/trn_rl_repo",):
    if os.path.isdir(_p) and _p not in sys.path:
        sys.path.insert(0, _p)

B, S, D = 2, 2048, 1024
HEADS, DH = 16, 64
N_CORES = 8
HPC = HEADS * B // N_CORES          # heads per core = 4
HC = HPC * DH                       # per-core head cols = 256
KT_N = D // 128                     # 8 contraction tiles for projections
TT_N = S // 128                     # 16 t-tiles
SC_W = 512                          # matmul free-dim (one PSUM bank)
AC_W = 1024                         # attention s-chunk width
AC_N = S // AC_W                    # 2 attention chunks

_CACHE = {}
LAST_RESULTS = None


def build_nc():
    """Build and compile the single-core SPMD Bass program."""
    import concourse.mybir as mybir
    import concourse.tile as tile
    from concourse import bacc
    from concourse.bass import ts, ds

    from concourse.masks import make_identity

    f32 = mybir.dt.float32
    bf16 = mybir.dt.bfloat16
    Relu = mybir.ActivationFunctionType.Relu
    Copy = mybir.ActivationFunctionType.Copy

    nc = bacc.Bacc("TRN2", target_bir_lowering=False, debug=False)

    xT_d = nc.dram_tensor("xT", (D, S), bf16, kind="ExternalInput")
    wq_d = nc.dram_tensor("wq", (D, HC), bf16, kind="ExternalInput")
    wkv_d = nc.dram_tensor("wkv", (D, HC), bf16, kind="ExternalInput")
    wout_d = nc.dram_tensor("wout", (HC, D), bf16, kind="ExternalInput")
    part_d = nc.dram_tensor("part", (S, D), f32, kind="ExternalOutput")

    with tile.TileContext(nc) as tc:
        with (
            tc.tile_pool(name="persist", bufs=1) as pp,
            tc.tile_pool(name="at", bufs=34) as atp,
            tc.tile_pool(name="osb", bufs=3) as outp,
        ):
            xt = pp.tile([128, KT_N, S], bf16)       # x.T, d on partitions
            wq = pp.tile([128, KT_N, HC], bf16)
            wkv = pp.tile([128, KT_N, HC], bf16)
            wout = pp.tile([128, 2, D], bf16)
            qt = pp.tile([128, 2, S], bf16)          # per pair: [2 heads*64, S]
            kt = pp.tile([128, 2, S], bf16)
            vt = pp.tile([128, TT_N, HC], bf16)      # t on partitions
            ot = pp.tile([128, 2, S], bf16)          # attention out, dh on part
            ident = pp.tile([128, 128], bf16)
            make_identity(nc, ident[:])

            # Contiguous k-major loads: each projection chain steps as its
            # k-tile lands. Weights go on the ACT HWDGE ring, xt alternates
            # rings, so per-dma issue overhead is paid in parallel.
            for k in range(KT_N):
                nc.gpsimd.dma_start(wq[:, k, :], wq_d[ts(k, 128), :])
                nc.gpsimd.dma_start(wkv[:, k, :], wkv_d[ts(k, 128), :])
                eng = nc.sync if k % 2 == 0 else nc.scalar
                eng.dma_start(xt[:, k, :], xT_d[ts(k, 128), :])
            for p in range(2):
                nc.gpsimd.dma_start(wout[:, p, :], wout_d[ts(p, 128), :])

            with tc.tile_pool(name="psS", bufs=4, space="PSUM") as psS:
              # ---- phase 1: projections, column-chunk-major ----
              with tc.tile_pool(name="psP", bufs=4, space="PSUM") as psP:

                def w_chain(w_sb, out_sb, eng, p, c):
                    ps = psP.tile([128, SC_W], f32, tag="m")
                    for k in range(KT_N):
                        nc.tensor.matmul(
                            ps[:],
                            w_sb[:, k, ts(p, 128)],
                            xt[:, k, ds(c * SC_W, SC_W)],
                            start=(k == 0),
                            stop=(k == KT_N - 1),
                        )
                    if eng == 0:
                        nc.scalar.activation(
                            out_sb[:, p, ds(c * SC_W, SC_W)], ps[:], Copy
                        )
                    else:
                        nc.vector.tensor_copy(out_sb[:, p, ds(c * SC_W, SC_W)], ps[:])

                def v_transpose(tt, p):
                    # v == k: build V t-tiles by PE-transposing kt (exact).
                    pst = psP.tile([128, 128], bf16, tag="m", name="pst")
                    nc.tensor.transpose(pst[:], kt[:, p, ts(tt, 128)], ident[:])
                    if (tt + p) % 2 == 0:
                        nc.scalar.activation(vt[:, tt, ts(p, 128)], pst[:], Copy)
                    else:
                        nc.vector.tensor_copy(vt[:, tt, ts(p, 128)], pst[:])

                for c in range(S // SC_W):
                    w_chain(wkv, kt, 1, 0, c)
                    w_chain(wq, qt, 0, 0, c)
                    w_chain(wkv, kt, 1, 1, c)
                    w_chain(wq, qt, 0, 1, c)
                    for j in range(4):
                        tt = 4 * c + j
                        v_transpose(tt, 0)
                        v_transpose(tt, 1)

              # ---- phase 2: software-pipelined attention ----
              # Chunks of the s-axis; the last chunk is split so the final
              # (non-overlappable) A@V + out-proj drain is short.
              with (
                tc.tile_pool(name="psO", bufs=2, space="PSUM") as psO,
                tc.tile_pool(name="psF", bufs=2, space="PSUM") as psF,
              ):
                chunks = [(0, 1024), (1024, 512), (1536, 512)]
                heads = [(ci, h) for ci in range(len(chunks)) for h in range(HPC)]
                state = None       # head currently in its A@V stage
                outproj = []       # pending (chunk, s-tile, nch) psum groups

                def emit_outproj_group(ci, st, nch, pool=None, tag="f"):
                    st0 = chunks[ci][0] + st * 128
                    psf = (pool or psF).tile([128, SC_W], f32, tag=tag, name="psf")
                    for p in range(2):
                        nc.tensor.matmul(
                            psf[:],
                            ot[:, p, ds(st0, 128)],
                            wout[:, p, ds(nch * SC_W, SC_W)],
                            start=(p == 0),
                            stop=(p == 1),
                        )
                    if nch == 0:
                        osb = outp.tile([128, D], f32, tag="osb", name="osb")
                        osb_tiles[st0] = osb
                    else:
                        osb = osb_tiles.pop(st0)
                    # split the psum->sbuf copy across both engines so the
                    # psf slot frees in half the time
                    half = SC_W // 2
                    o0 = nch * SC_W
                    nc.scalar.activation(osb[:, ds(o0, half)], psf[:, ds(0, half)], Copy)
                    nc.vector.tensor_copy(
                        osb[:, ds(o0 + half, half)], psf[:, ds(half, half)]
                    )
                    if nch == 1:
                        nc.sync.dma_start(part_d[ds(st0, 128), :], osb[:])

                osb_tiles = {}
                for i in range(len(heads) + 1):
                    cur = heads[i] if i < len(heads) else None
                    if cur is not None:
                        ci, h = cur
                        s0, cw = chunks[ci]
                        ws = [SC_W] * (cw // SC_W) if cw >= SC_W else [cw]
                        nsh = len(ws)
                        p, half = divmod(h, 2)
                        r0 = half * 64
                        ats = []
                    if state is not None:
                        s_s0, s_cw = chunks[state["ci"]]
                        s_ws = [SC_W] * (s_cw // SC_W) if s_cw >= SC_W else [s_cw]
                        s_nsh = len(s_ws)
                        pso = [
                            psO.tile([64, s_ws[sh]], f32, tag="o", name=f"pso{sh}")
                            for sh in range(s_nsh)
                        ]
                    # t-tile blocks: a burst of scores matmuls (fills the 4
                    # psS slots) then a burst of A@V matmuls — minimizes the
                    # PE array-reconfig switches vs per-t-tile interleave.
                    BL = (4 // nsh) if cur is not None else 4
                    for bt in range(0, TT_N, BL):
                        if cur is not None:
                            for tt in range(bt, bt + BL):
                                psas = []
                                for sh in range(nsh):
                                    psa = psS.tile([128, ws[sh]], f32, tag="s")
                                    nc.tensor.matmul(
                                        psa[:],
                                        kt[r0 : r0 + 64, p, ts(tt, 128)],
                                        qt[r0 : r0 + 64, p, ds(s0 + sh * SC_W, ws[sh])],
                                        start=True,
                                        stop=True,
                                    )
                                    psas.append(psa)
                                at = atp.tile([128, cw], bf16, tag="at")
                                for sh in range(nsh):
                                    dst = at[:, ds(sh * SC_W, ws[sh])]
                                    if (sh + tt) % 2 == 0:
                                        nc.scalar.activation(dst, psas[sh][:], Relu)
                                    else:
                                        nc.vector.tensor_scalar_max(
                                            dst, psas[sh][:], 0.0
                                        )
                                ats.append(at)
                        if state is not None:
                            sh_h = state["h"]
                            for tt in range(bt, bt + BL):
                                for sh in range(s_nsh):
                                    nc.tensor.matmul(
                                        pso[sh][:],
                                        vt[:, tt, ds(sh_h * DH, DH)],
                                        state["ats"][tt][:, ds(sh * SC_W, s_ws[sh])],
                                        start=(tt == 0),
                                        stop=(tt == TT_N - 1),
                                    )
                        if outproj:
                            emit_outproj_group(*outproj.pop(0))
                    if state is not None:
                        sh_h = state["h"]
                        sp, shalf = divmod(sh_h, 2)
                        sr0 = shalf * 64
                        for sh in range(s_nsh):
                            w = s_ws[sh]
                            o0 = s_s0 + sh * SC_W
                            if s_nsh == 1:
                                # split across both engines: pso frees sooner
                                nc.scalar.activation(
                                    ot[sr0 : sr0 + 64, sp, ds(o0, w // 2)],
                                    pso[sh][:, ds(0, w // 2)],
                                    Copy,
                                )
                                nc.vector.tensor_copy(
                                    ot[sr0 : sr0 + 64, sp, ds(o0 + w // 2, w // 2)],
                                    pso[sh][:, ds(w // 2, w // 2)],
                                )
                            elif (sh + sh_h) % 2 == 0:
                                nc.scalar.activation(
                                    ot[sr0 : sr0 + 64, sp, ds(o0, w)], pso[sh][:], Copy
                                )
                            else:
                                nc.vector.tensor_copy(
                                    ot[sr0 : sr0 + 64, sp, ds(o0, w)], pso[sh][:]
                                )
                        if sh_h == HPC - 1:
                            for st in range(s_cw // 128):
                                for nch in range(2):
                                    outproj.append((state["ci"], st, nch))
                    if cur is not None:
                        state = {"ci": ci, "h": h, "ats": ats}
                    else:
                        state = None
                # drain the last chunk's output projection
                while outproj:
                    emit_outproj_group(*outproj.pop(0))

    nc.compile()
    return nc


def _get_nc():
    if "nc" not in _CACHE:
        _CACHE["nc"] = build_nc()
    return _CACHE["nc"]


def make_in_maps(hidden_states, Wq, Wkv, Wout):
    bf = ml_dtypes.bfloat16
    x = np.asarray(hidden_states, dtype=np.float32)
    Wq = np.asarray(Wq, dtype=np.float32)
    Wkv = np.asarray(Wkv, dtype=np.float32)
    Wout = np.asarray(Wout, dtype=np.float32)
    scale = 1.0 / np.sqrt(np.float32(DH))
    xT = [np.ascontiguousarray(x[b].T).astype(bf) for b in range(B)]
    in_maps = []
    for c in range(N_CORES):
        b, j = divmod(c, N_CORES // B)
        h0 = j * HC
        in_maps.append(
            {
                "xT": xT[b],
                "wq": np.ascontiguousarray(Wq[:, h0 : h0 + HC] * scale).astype(bf),
                "wkv": np.ascontiguousarray(Wkv[:, h0 : h0 + HC]).astype(bf),
                "wout": np.ascontiguousarray(Wout[h0 : h0 + HC, :]).astype(bf),
            }
        )
    return in_maps


def kernel(**inputs):
    global LAST_RESULTS
    from concourse.bass_utils import run_bass_kernel_spmd

    nc = _get_nc()
    in_maps = make_in_maps(
        inputs["hidden_states"], inputs["Wq"], inputs["Wkv"], inputs["Wout"]
    )
    trace = bool(os.environ.get("KERNEL_TRACE"))
    res = run_bass_kernel_spmd(
        nc, in_maps, core_ids=list(range(N_CORES)), trace=trace
    )
    LAST_RESULTS = res
    out = np.zeros((B, S, D), dtype=np.float32)
    for c in range(N_CORES):
        out[c // (N_CORES // B)] += res.results[c]["part"]
    out += np.asarray(inputs["bout"], dtype=np.float32)[None, None, :]
    return out
